# revision 3
# baseline (speedup 1.0000x reference)
"""Trainium2 Bass kernel for CausalCrossConditionalSelfAttention.

Reference semantics (B=2, T=2560, C=768, H=12, hd=64, t=T//10=256):
  q/k/v = x @ W{q,k,v}.T + b{q,k,v}           (per-head slices of C)
  att   = softmax(mask(q k^T / sqrt(hd)))      mask: (i%256) >= (j%256)
  y     = (att @ v) @ Wp.T + bp

Sharding: 8 cores = 2 batches x 4 head-groups (3 heads each).
Each core computes its (batch, 3 heads) slab fully on-chip and returns a
partial pre-projection output out^T [768, 2560]; the host sums the 4
head-group partials per batch and adds the constant bias (bp + Wp @ bv).

Device-side layout (per core):
  xT      [768, 2560]  x[b].T
  wqk     [768, 384]   cols: [Qh0|Qh1 | Kh0|Kh1 | Qh2 | Kh2] weight.T cols
  bqk     [4, 128, 1]  per-partition bias rows for the 4 col-groups
  wv      [768, 192]   Wv rows for the 3 heads, transposed
  wp      [3, 64, 768] per-head Wp[:, head_slice].T
  out     [768, 2560]  partial out^T (pre-bias)

The scores are computed transposed: S^T[k, q] in PSUM, exp'd on ScalarE
(scale=1/8 fused), masked by GPSIMD affine_select (exact zeros), and
contracted with V (ones column appended -> softmax denominator for free).
The (k%256)>=128 x (q%256)<128 quarter of each 256x256 mask block is fully
masked and skipped entirely (25% of score/AV/exp work).
"""

import numpy as np

B, T, C = 2, 2560, 768
H, HD = 12, 64
HPG = 3            # heads per group (core)
CW = HPG * HD      # 192
NKC = T // 128     # 20 key chunks of 128
NQT = T // 512     # 5 query tiles of 512
N_CORES = 8

_CACHE = {}


def _split_multi_waits(nc, maxw=1):
    """walrus in this container rejects >1 sync wait per instruction;
    split extra waits onto preceding NOPs on the same engine."""
    import concourse.mybir as mybir
    for f in nc.m.functions:
        for bb in f.blocks:
            newlist = []
            for ins in bb.instructions:
                si = ins.sync_info
                if si is not None and si.on_wait and len(si.on_wait) > maxw:
                    waits = list(si.on_wait)
                    chunks = [waits[i:i + maxw] for i in range(0, len(waits), maxw)]
                    for ch in chunks[:-1]:
                        newlist.append(mybir.InstNoOp(
                            name=f"WSPLIT-{nc.next_id()}",
                            engine=ins.engine,
                            sync_info=mybir.SyncInfo(on_wait=list(ch), on_update=[]),
                            text_hint="wait_split",
                        ))
                    ins.sync_info = mybir.SyncInfo(
                        on_wait=list(chunks[-1]), on_update=list(si.on_update))
                newlist.append(ins)
            bb.instructions = newlist
    return nc


def _chunks(lst, n):
    return [lst[i:i + n] for i in range(0, len(lst), n)]


def build_program():
    import concourse.bass as bass
    import concourse.mybir as mybir
    import concourse.tile as tile

    f32 = mybir.dt.float32
    AF = mybir.ActivationFunctionType
    ALU = mybir.AluOpType

    nc = bass.Bass()
    xT = nc.dram_tensor("xT", [C, T], f32, kind="ExternalInput")
    wqk = nc.dram_tensor("wqk", [C, 384], f32, kind="ExternalInput")
    bqk = nc.dram_tensor("bqk", [4, 128, 1], f32, kind="ExternalInput")
    wv = nc.dram_tensor("wv", [C, CW], f32, kind="ExternalInput")
    wp = nc.dram_tensor("wp", [HPG, HD, C], f32, kind="ExternalInput")
    out = nc.dram_tensor("out", [C, T], f32, kind="ExternalOutput")

    with tile.TileContext(nc) as tc:
        with tc.tile_pool(name="persist", bufs=1) as persist, \
             tc.tile_pool(name="work", bufs=2) as work, \
             tc.tile_pool(name="psum", bufs=2, space="PSUM") as psum:

            # ---------------- load inputs ----------------
            xt_sb = persist.tile([128, 6, T], f32)       # x^T, 6 chunks of C
            for c in range(6):
                for qt in range(NQT):
                    nc.sync.dma_start(
                        out=xt_sb[:, c, qt * 512:(qt + 1) * 512],
                        in_=xT[c * 128:(c + 1) * 128, qt * 512:(qt + 1) * 512])
            wqk_sb = persist.tile([128, 6, 384], f32)
            for c in range(6):
                nc.sync.dma_start(out=wqk_sb[:, c, :],
                                  in_=wqk[c * 128:(c + 1) * 128, :])
            wv_sb = persist.tile([128, 6, CW], f32)
            for c in range(6):
                nc.sync.dma_start(out=wv_sb[:, c, :],
                                  in_=wv[c * 128:(c + 1) * 128, :])
            wp_sb = persist.tile([64, HPG, C], f32)
            for h in range(HPG):
                nc.sync.dma_start(out=wp_sb[:, h, :], in_=wp[h])
            bqk_sb = persist.tile([128, 4, 1], f32)
            for j in range(4):
                nc.sync.dma_start(out=bqk_sb[:, j, :], in_=bqk[j])

            ones_sb = persist.tile([128, 64], f32)
            nc.vector.memset(ones_sb, 1.0)

            # ---------------- q/k projections (transposed) ----------------
            # qkT j0=[Qh0|Qh1] j1=[Kh0|Kh1] (128 parts); j2=Qh2 j3=Kh2 (64)
            qkT01q = persist.tile([128, T], f32)
            qkT01k = persist.tile([128, T], f32)
            qkT2q = persist.tile([64, T], f32)
            qkT2k = persist.tile([64, T], f32)
            jdefs = [(qkT01q, 0, 128), (qkT01k, 128, 128),
                     (qkT2q, 256, 64), (qkT2k, 320, 64)]
            for j, (dst, col0, m) in enumerate(jdefs):
                for qt in range(NQT):
                    qk_ps = psum.tile([128, 512], f32, tag="av", name="qk_ps")
                    for c in range(6):
                        nc.tensor.matmul(
                            qk_ps[:m, :],
                            lhsT=wqk_sb[:, c, col0:col0 + m],
                            rhs=xt_sb[:, c, qt * 512:(qt + 1) * 512],
                            start=(c == 0), stop=(c == 5))
                    nc.vector.tensor_scalar_add(
                        dst[:m, qt * 512:(qt + 1) * 512],
                        qk_ps[:m, :], bqk_sb[:m, j, :])

            # ---------------- v projection (natural layout + ones col) ----
            # per head h: cols [65h .. 65h+63] = V_h, col 65h+64 = 1.0
            v_sb = persist.tile([128, NKC, HPG * 65], f32)
            v_r = v_sb.rearrange("p n (h c) -> p n h c", c=65)
            nc.vector.memset(v_r[:, :, :, 64], 1.0)
            for tch in range(NKC):
                v_ps = psum.tile([128, 512], f32, tag="av", name="v_ps")
                for c in range(6):
                    nc.tensor.matmul(
                        v_ps[:, :CW],
                        lhsT=xt_sb[:, c, tch * 128:(tch + 1) * 128],
                        rhs=wv_sb[:, c, :],
                        start=(c == 0), stop=(c == 5))
                nc.vector.tensor_copy(
                    v_r[:, tch, :, 0:64],
                    v_ps[:, :CW].rearrange("p (h c) -> p h c", h=HPG))

            # ---------------- attention ----------------
            evens = list(range(0, NKC, 2))
            odds = list(range(1, NKC, 2))
            ynorm = [persist.tile([64, T], f32, name=f"ynorm{h}")
                     for h in range(HPG)]

            for h in range(HPG):
                if h < 2:
                    qTh = qkT01q[64 * h:64 * (h + 1), :]
                    kTh = qkT01k[64 * h:64 * (h + 1), :]
                else:
                    qTh = qkT2q[0:64, :]
                    kTh = qkT2k[0:64, :]
                # odd-subchunk view of q: [64, qt, two, sp, 128]
                q_odd = qTh.rearrange("p (q s t c) -> p q t s c",
                                      q=NQT, s=2, t=2, c=128)

                for qt in range(NQT):
                    qwin = qTh[:, qt * 512:(qt + 1) * 512]
                    av = psum.tile([128, 512], f32, tag="av", name="av")
                    av_odd = av.rearrange("p (s t c) -> p t s c",
                                          s=2, t=2, c=128)[:, 1]

                    for grp in _chunks(evens, 3):
                        L = len(grp)
                        sc = psum.tile([128, 1536], f32, tag="sc", name="sc")
                        for i, kc in enumerate(grp):
                            nc.tensor.matmul(
                                sc[:, i * 512:(i + 1) * 512],
                                lhsT=kTh[:, kc * 128:(kc + 1) * 128],
                                rhs=qwin, start=True, stop=True)
                        pt = work.tile([128, 1536], f32, tag="pt", name="pt")
                        nc.scalar.activation(pt[:, :L * 512], sc[:, :L * 512],
                                             AF.Exp, scale=0.125)
                        pt_tri = pt[:, :L * 512].rearrange(
                            "p (l s t c) -> p l t s c", l=L, s=2, t=2, c=128)[:, :, 0]
                        nc.gpsimd.affine_select(
                            out=pt_tri, in_=pt_tri,
                            pattern=[[0, L], [0, 2], [1, 128]],
                            channel_multiplier=-1, base=0,
                            compare_op=ALU.is_ge, fill=0.0)
                        for i, kc in enumerate(grp):
                            nc.tensor.matmul(
                                av[:65, :],
                                lhsT=v_sb[:, kc, 65 * h:65 * h + 65],
                                rhs=pt[:, i * 512:(i + 1) * 512],
                                start=(kc == 0), stop=False,
                                skip_group_check=True)

                    for gi, grp in enumerate(_chunks(odds, 3)):
                        L = len(grp)
                        last_grp = (gi == 3)
                        sc = psum.tile([128, 1536], f32, tag="sc", name="sc")
                        for i, kc in enumerate(grp):
                            nc.tensor.matmul(
                                sc[:, i * 256:(i + 1) * 256],
                                lhsT=kTh[:, kc * 128:(kc + 1) * 128],
                                rhs=q_odd[:, qt, 1], start=True, stop=True)
                        pt = work.tile([128, 1536], f32, tag="pt", name="pt")
                        nc.scalar.activation(pt[:, :L * 256], sc[:, :L * 256],
                                             AF.Exp, scale=0.125)
                        pt_tri = pt[:, :L * 256].rearrange(
                            "p (l s c) -> p l s c", l=L, s=2, c=128)
                        nc.gpsimd.affine_select(
                            out=pt_tri, in_=pt_tri,
                            pattern=[[0, L], [0, 2], [1, 128]],
                            channel_multiplier=-1, base=0,
                            compare_op=ALU.is_ge, fill=0.0)
                        for i, kc in enumerate(grp):
                            nc.tensor.matmul(
                                av_odd[:65],
                                lhsT=v_sb[:, kc, 65 * h:65 * h + 65],
                                rhs=pt[:, i * 256:(i + 1) * 256],
                                start=False, stop=(kc == NKC - 1),
                                skip_group_check=True)

                    # normalize: y = av[0:64] / av[64]  (denominator row)
                    rcp = work.tile([65, 512], f32, tag="rcp", name="rcp")
                    nc.vector.reciprocal(rcp[64:65, :], av[64:65, :])
                    bc_ps = psum.tile([128, 512], f32, tag="av", name="bc_ps")
                    nc.tensor.matmul(bc_ps[:64, :],
                                     lhsT=ones_sb[64:65, :],
                                     rhs=rcp[64:65, :],
                                     start=True, stop=True)
                    bc_sb = work.tile([64, 512], f32, tag="bc", name="bc_sb")
                    nc.vector.tensor_copy(bc_sb, bc_ps[:64, :])
                    nc.vector.tensor_mul(
                        ynorm[h][:, qt * 512:(qt + 1) * 512],
                        av[0:64, :], bc_sb)

            # ---------------- output projection ----------------
            for m in range(6):
                for qt in range(NQT):
                    pj_ps = psum.tile([128, 512], f32, tag="av", name="pj_ps")
                    for h in range(HPG):
                        nc.tensor.matmul(
                            pj_ps,
                            lhsT=wp_sb[:, h, m * 128:(m + 1) * 128],
                            rhs=ynorm[h][:, qt * 512:(qt + 1) * 512],
                            start=(h == 0), stop=(h == 2))
                    pj_sb = work.tile([128, 512], f32, tag="pj", name="pj_sb")
                    nc.vector.tensor_copy(pj_sb, pj_ps)
                    nc.sync.dma_start(
                        out=out[m * 128:(m + 1) * 128, qt * 512:(qt + 1) * 512],
                        in_=pj_sb)

    _split_multi_waits(nc)
    return nc


def get_program():
    if "nc" not in _CACHE:
        _CACHE["nc"] = build_program()
    return _CACHE["nc"]


def make_in_maps(x, Wk, bk, Wq, bq, Wv, bv, Wp, bp):
    x = np.asarray(x, dtype=np.float32)
    in_maps = []
    for core in range(N_CORES):
        b, g = divmod(core, 4)
        h0 = g * HPG
        r = slice(h0 * HD, (h0 + HPG) * HD)     # 192 head dims
        xt = np.ascontiguousarray(x[b].T)
        wq_g = np.asarray(Wq)[r]                 # [192, 768]
        wk_g = np.asarray(Wk)[r]
        # wqk cols: [Qh0|Qh1(128) | Kh0|Kh1(128) | Qh2(64) | Kh2(64)]
        wqk = np.concatenate(
            [wq_g[:128].T, wk_g[:128].T, wq_g[128:].T, wk_g[128:].T],
            axis=1).astype(np.float32)
        bq_g = np.asarray(bq)[r].astype(np.float32)
        bk_g = np.asarray(bk)[r].astype(np.float32)
        bqk = np.zeros((4, 128, 1), np.float32)
        bqk[0, :, 0] = bq_g[:128]
        bqk[1, :, 0] = bk_g[:128]
        bqk[2, :64, 0] = bq_g[128:]
        bqk[3, :64, 0] = bk_g[128:]
        wv_g = np.ascontiguousarray(np.asarray(Wv)[r].T).astype(np.float32)
        wp_g = np.asarray(Wp)[:, r]              # [768, 192]
        wp_t = np.ascontiguousarray(
            wp_g.T.reshape(HPG, HD, C)).astype(np.float32)
        in_maps.append({
            "xT": np.ascontiguousarray(xt),
            "wqk": np.ascontiguousarray(wqk),
            "bqk": bqk,
            "wv": wv_g,
            "wp": wp_t,
        })
    return in_maps


def kernel(x, Wk, bk, Wq, bq, Wv, bv, Wp, bp):
    from concourse.bass_utils import run_bass_kernel_spmd
    nc = get_program()
    in_maps = make_in_maps(x, Wk, bk, Wq, bq, Wv, bv, Wp, bp)
    res = run_bass_kernel_spmd(nc, in_maps, list(range(N_CORES)))
    Wp_np = np.asarray(Wp, dtype=np.float32)
    const = (np.asarray(bp, dtype=np.float32)
             + Wp_np @ np.asarray(bv, dtype=np.float32))   # [768]
    out = np.empty((B, T, C), dtype=np.float32)
    for b in range(B):
        acc = res.results[b * 4 + 0]["out"].astype(np.float32).copy()
        for g in range(1, 4):
            acc += res.results[b * 4 + g]["out"]
        out[b] = acc.T + const[None, :]
    return out


# revision 8
# speedup vs baseline: 1.3889x; 1.3889x over previous
"""Trainium2 Bass kernel for CausalCrossConditionalSelfAttention.

Reference semantics (B=2, T=2560, C=768, H=12, hd=64, t=T//10=256):
  q/k/v = x @ W{q,k,v}.T + b{q,k,v}           (per-head slices of C)
  att   = softmax(mask(q k^T / sqrt(hd)))      mask: (i%256) >= (j%256)
  y     = (att @ v) @ Wp.T + bp

Sharding: 8 cores = 2 batches x 4 head-groups (3 heads each).
Each core computes its (batch, 3 heads) slab fully on-chip and returns a
partial pre-projection output out^T [768, 2560]; the host sums the 4
head-group partials per batch and adds the constant bias (bp + Wp @ bv).

Device-side layout (per core):
  xT      [768, 2560]  x[b].T
  wqk     [768, 384]   cols: [Qh0|Qh1 | Kh0|Kh1 | Qh2 | Kh2] weight.T cols
  bqk     [4, 128, 1]  per-partition bias rows for the 4 col-groups
  wv      [768, 192]   Wv rows for the 3 heads, transposed
  wp      [3, 64, 768] per-head Wp[:, head_slice].T
  out     [768, 2560]  partial out^T (pre-bias)

The scores are computed transposed: S^T[k, q] in PSUM, exp'd on ScalarE
(scale=1/8 fused), masked by GPSIMD affine_select (exact zeros), and
contracted with V (ones column appended -> softmax denominator for free).
The (k%256)>=128 x (q%256)<128 quarter of each 256x256 mask block is fully
masked and skipped entirely (25% of score/AV/exp work).
"""

import numpy as np

B, T, C = 2, 2560, 768
H, HD = 12, 64
HPG = 3            # heads per group (core)
CW = HPG * HD      # 192
NKC = T // 128     # 20 key chunks of 128
NQT = T // 512     # 5 query tiles of 512
N_CORES = 8

_CACHE = {}


def _split_multi_waits(nc, maxw=1):
    """walrus in this container rejects >1 sync wait per instruction;
    split extra waits onto preceding NOPs on the same engine."""
    import concourse.mybir as mybir
    for f in nc.m.functions:
        for bb in f.blocks:
            newlist = []
            for ins in bb.instructions:
                si = ins.sync_info
                if si is not None and si.on_wait and len(si.on_wait) > maxw:
                    waits = list(si.on_wait)
                    chunks = [waits[i:i + maxw] for i in range(0, len(waits), maxw)]
                    for ch in chunks[:-1]:
                        newlist.append(mybir.InstNoOp(
                            name=f"WSPLIT-{nc.next_id()}",
                            engine=ins.engine,
                            sync_info=mybir.SyncInfo(on_wait=list(ch), on_update=[]),
                            text_hint="wait_split",
                        ))
                    ins.sync_info = mybir.SyncInfo(
                        on_wait=list(chunks[-1]), on_update=list(si.on_update))
                newlist.append(ins)
            bb.instructions = newlist
    return nc


def _chunks(lst, n):
    return [lst[i:i + n] for i in range(0, len(lst), n)]


def build_program():
    import concourse.bass as bass
    import concourse.mybir as mybir
    import concourse.tile as tile

    f32 = mybir.dt.float32
    bf16 = mybir.dt.bfloat16
    AF = mybir.ActivationFunctionType
    ALU = mybir.AluOpType

    nc = bass.Bass()
    xT = nc.dram_tensor("xT", [C, T], f32, kind="ExternalInput")
    wqk = nc.dram_tensor("wqk", [C, 384], f32, kind="ExternalInput")
    bqk = nc.dram_tensor("bqk", [4, 128, 1], f32, kind="ExternalInput")
    wv = nc.dram_tensor("wv", [C, CW], f32, kind="ExternalInput")
    wp = nc.dram_tensor("wp", [HPG, HD, C], f32, kind="ExternalInput")
    identm = nc.dram_tensor("identm", [128, 384], f32, kind="ExternalInput")
    out = nc.dram_tensor("out", [C, T], f32, kind="ExternalOutput")

    with tile.TileContext(nc) as tc:
        with tc.tile_pool(name="persist", bufs=1) as persist, \
             tc.tile_pool(name="work", bufs=2) as work, \
             tc.tile_pool(name="psum", bufs=2, space="PSUM") as psum:

            # ---------------- load inputs ----------------
            xt_sb = persist.tile([128, 6, T], f32)       # x^T, 6 chunks of C
            for c in range(6):
                for qt in range(NQT):
                    nc.sync.dma_start(
                        out=xt_sb[:, c, qt * 512:(qt + 1) * 512],
                        in_=xT[c * 128:(c + 1) * 128, qt * 512:(qt + 1) * 512])
            wqk_sb = persist.tile([128, 6, 384], f32)
            for c in range(6):
                nc.sync.dma_start(out=wqk_sb[:, c, :],
                                  in_=wqk[c * 128:(c + 1) * 128, :])
            wv_sb = persist.tile([128, 6, CW], f32)
            for c in range(6):
                nc.sync.dma_start(out=wv_sb[:, c, :],
                                  in_=wv[c * 128:(c + 1) * 128, :])
            wp_sb = persist.tile([64, HPG, C], f32)
            for h in range(HPG):
                nc.sync.dma_start(out=wp_sb[:, h, :], in_=wp[h])
            bqk_sb = persist.tile([128, 4, 1], f32)
            for j in range(4):
                nc.sync.dma_start(out=bqk_sb[:, j, :], in_=bqk[j])

            ones_sb = persist.tile([128, 64], f32)
            nc.vector.memset(ones_sb, 1.0)
            identm_f = work.tile([128, 384], f32, tag="im", bufs=1, name="identm_f")
            nc.sync.dma_start(out=identm_f, in_=identm[:, :])
            ident_sb = persist.tile([128, 128], bf16)   # identity
            maskm_sb = persist.tile([128, 256], bf16)   # [L|L], L=-1e9 if j<i
            nc.vector.tensor_copy(ident_sb, identm_f[:, 0:128])
            nc.vector.tensor_copy(maskm_sb, identm_f[:, 128:384])

            # ---------------- q/k projections (transposed) ----------------
            # qkT j0=[Qh0|Qh1] j1=[Kh0|Kh1] (128 parts); j2=Qh2 j3=Kh2 (64)
            qkT01q = persist.tile([128, T], bf16)
            qkT01k = persist.tile([128, T], bf16)
            qkT2q = persist.tile([64, T], bf16)
            qkT2k = persist.tile([64, T], bf16)
            jdefs = [(qkT01q, 0, 128), (qkT01k, 128, 128),
                     (qkT2q, 256, 64), (qkT2k, 320, 64)]
            for j, (dst, col0, m) in enumerate(jdefs):
                for qt in range(NQT):
                    qk_ps = psum.tile([128, 512], f32, tag="av", name="qk_ps")
                    for c in range(6):
                        nc.tensor.matmul(
                            qk_ps[:m, :],
                            lhsT=wqk_sb[:, c, col0:col0 + m],
                            rhs=xt_sb[:, c, qt * 512:(qt + 1) * 512],
                            start=(c == 0), stop=(c == 5))
                    nc.vector.tensor_scalar_add(
                        dst[:m, qt * 512:(qt + 1) * 512],
                        qk_ps[:m, :], bqk_sb[:m, j, :])

            # ---------------- v projection (natural layout + ones col) ----
            # per head h: cols [65h .. 65h+63] = V_h, col 65h+64 = 1.0
            v_sb = persist.tile([128, NKC, HPG * 65], bf16)
            v_r = v_sb.rearrange("p n (h c) -> p n h c", c=65)
            nc.vector.memset(v_r[:, :, :, 64], 1.0)
            for tch in range(NKC):
                v_ps = psum.tile([128, 512], f32, tag="av", name="v_ps")
                for c in range(6):
                    nc.tensor.matmul(
                        v_ps[:, :CW],
                        lhsT=xt_sb[:, c, tch * 128:(tch + 1) * 128],
                        rhs=wv_sb[:, c, :],
                        start=(c == 0), stop=(c == 5))
                nc.vector.tensor_copy(
                    v_r[:, tch, :, 0:64],
                    v_ps[:, :CW].rearrange("p (h c) -> p h c", h=HPG))

            # ---------------- attention ----------------
            evens = list(range(0, NKC, 2))
            odds = list(range(1, NKC, 2))
            ynorm = [persist.tile([64, T], f32, name=f"ynorm{h}")
                     for h in range(HPG)]

            for h in range(HPG):
                if h < 2:
                    qTh = qkT01q[64 * h:64 * (h + 1), :]
                    kTh = qkT01k[64 * h:64 * (h + 1), :]
                else:
                    qTh = qkT2q[0:64, :]
                    kTh = qkT2k[0:64, :]
                # odd-subchunk view of q: [64, qt, two, sp, 128]
                q_odd = qTh.rearrange("p (q s t c) -> p q t s c",
                                      q=NQT, s=2, t=2, c=128)

                for qt in range(NQT):
                    qwin = qTh[:, qt * 512:(qt + 1) * 512]
                    av = psum.tile([128, 512], f32, tag="av", name="av")
                    av_odd = av.rearrange("p (s t c) -> p t s c",
                                          s=2, t=2, c=128)[:, 1]

                    for grp in _chunks(evens, 3):
                        L = len(grp)
                        sc = psum.tile([128, 1536], f32, tag="sc", name="sc")
                        for i, kc in enumerate(grp):
                            nc.tensor.matmul(
                                sc[:, i * 512:(i + 1) * 512],
                                lhsT=kTh[:, kc * 128:(kc + 1) * 128],
                                rhs=qwin, start=True, stop=False,
                                skip_group_check=True)
                        sc_r = sc.rearrange("p (l s t c) -> p l s t c",
                                            l=3, s=2, t=2, c=128)
                        for i in range(L):
                            nc.tensor.matmul(
                                sc_r[:, i, :, 0], lhsT=ident_sb, rhs=maskm_sb,
                                start=False, stop=True, skip_group_check=True)
                        pt = work.tile([128, 1536], bf16, tag="pt", name="pt")
                        nc.scalar.activation(pt[:, :L * 512], sc[:, :L * 512],
                                             AF.Exp, scale=0.125)
                        for i, kc in enumerate(grp):
                            nc.tensor.matmul(
                                av[:65, :],
                                lhsT=v_sb[:, kc, 65 * h:65 * h + 65],
                                rhs=pt[:, i * 512:(i + 1) * 512],
                                start=(kc == 0), stop=False,
                                skip_group_check=True)

                    for gi, grp in enumerate(_chunks(odds, 3)):
                        L = len(grp)
                        last_grp = (gi == 3)
                        sc = psum.tile([128, 1536], f32, tag="sc", name="sc")
                        for i, kc in enumerate(grp):
                            # 256-wide blocks: two share a 2KB psum bank, and
                            # start=True zero-marks the WHOLE bank - only the
                            # first block of each bank may set it.
                            nc.tensor.matmul(
                                sc[:, i * 256:(i + 1) * 256],
                                lhsT=kTh[:, kc * 128:(kc + 1) * 128],
                                rhs=q_odd[:, qt, 1], start=(i % 2 == 0),
                                stop=False, skip_group_check=True)
                        for i in range(L):
                            nc.tensor.matmul(
                                sc[:, i * 256:(i + 1) * 256],
                                lhsT=ident_sb, rhs=maskm_sb,
                                start=False, stop=True, skip_group_check=True)
                        pt = work.tile([128, 1536], bf16, tag="pt", name="pt")
                        nc.scalar.activation(pt[:, :L * 256], sc[:, :L * 256],
                                             AF.Exp, scale=0.125)
                        for i, kc in enumerate(grp):
                            nc.tensor.matmul(
                                av_odd[:65],
                                lhsT=v_sb[:, kc, 65 * h:65 * h + 65],
                                rhs=pt[:, i * 256:(i + 1) * 256],
                                start=False, stop=(kc == NKC - 1),
                                skip_group_check=True)

                    # normalize: y = av[0:64] / av[64]  (denominator row)
                    rcp = work.tile([65, 512], f32, tag="rcp", name="rcp")
                    nc.vector.reciprocal(rcp[64:65, :], av[64:65, :])
                    bc_ps = psum.tile([128, 512], f32, tag="av", name="bc_ps")
                    nc.tensor.matmul(bc_ps[:64, :],
                                     lhsT=ones_sb[64:65, :],
                                     rhs=rcp[64:65, :],
                                     start=True, stop=True)
                    bc_sb = work.tile([64, 512], f32, tag="bc", name="bc_sb")
                    nc.vector.tensor_copy(bc_sb, bc_ps[:64, :])
                    nc.vector.tensor_mul(
                        ynorm[h][:, qt * 512:(qt + 1) * 512],
                        av[0:64, :], bc_sb)

            # ---------------- output projection ----------------
            for m in range(6):
                for qt in range(NQT):
                    pj_ps = psum.tile([128, 512], f32, tag="av", name="pj_ps")
                    for h in range(HPG):
                        nc.tensor.matmul(
                            pj_ps,
                            lhsT=wp_sb[:, h, m * 128:(m + 1) * 128],
                            rhs=ynorm[h][:, qt * 512:(qt + 1) * 512],
                            start=(h == 0), stop=(h == 2))
                    pj_sb = work.tile([128, 512], f32, tag="pj", name="pj_sb")
                    nc.vector.tensor_copy(pj_sb, pj_ps)
                    nc.sync.dma_start(
                        out=out[m * 128:(m + 1) * 128, qt * 512:(qt + 1) * 512],
                        in_=pj_sb)

    _split_multi_waits(nc)
    return nc


def get_program():
    if "nc" not in _CACHE:
        _CACHE["nc"] = build_program()
    return _CACHE["nc"]


def make_in_maps(x, Wk, bk, Wq, bq, Wv, bv, Wp, bp):
    x = np.asarray(x, dtype=np.float32)
    in_maps = []
    for core in range(N_CORES):
        b, g = divmod(core, 4)
        h0 = g * HPG
        r = slice(h0 * HD, (h0 + HPG) * HD)     # 192 head dims
        xt = np.ascontiguousarray(x[b].T)
        wq_g = np.asarray(Wq)[r]                 # [192, 768]
        wk_g = np.asarray(Wk)[r]
        # wqk cols: [Qh0|Qh1(128) | Kh0|Kh1(128) | Qh2(64) | Kh2(64)]
        wqk = np.concatenate(
            [wq_g[:128].T, wk_g[:128].T, wq_g[128:].T, wk_g[128:].T],
            axis=1).astype(np.float32)
        bq_g = np.asarray(bq)[r].astype(np.float32)
        bk_g = np.asarray(bk)[r].astype(np.float32)
        bqk = np.zeros((4, 128, 1), np.float32)
        bqk[0, :, 0] = bq_g[:128]
        bqk[1, :, 0] = bk_g[:128]
        bqk[2, :64, 0] = bq_g[128:]
        bqk[3, :64, 0] = bk_g[128:]
        wv_g = np.ascontiguousarray(np.asarray(Wv)[r].T).astype(np.float32)
        wp_g = np.asarray(Wp)[:, r]              # [768, 192]
        wp_t = np.ascontiguousarray(
            wp_g.T.reshape(HPG, HD, C)).astype(np.float32)
        ident = np.eye(128, dtype=np.float32)
        L = np.where(np.arange(256)[None, :] % 128 < np.arange(128)[:, None],
                     np.float32(-1e9), np.float32(0.0))
        identm = np.concatenate([ident, L], axis=1).astype(np.float32)
        in_maps.append({
            "identm": identm,
            "xT": np.ascontiguousarray(xt),
            "wqk": np.ascontiguousarray(wqk),
            "bqk": bqk,
            "wv": wv_g,
            "wp": wp_t,
        })
    return in_maps


def kernel(x, Wk, bk, Wq, bq, Wv, bv, Wp, bp):
    from concourse.bass_utils import run_bass_kernel_spmd
    nc = get_program()
    in_maps = make_in_maps(x, Wk, bk, Wq, bq, Wv, bv, Wp, bp)
    res = run_bass_kernel_spmd(nc, in_maps, list(range(N_CORES)))
    Wp_np = np.asarray(Wp, dtype=np.float32)
    const = (np.asarray(bp, dtype=np.float32)
             + Wp_np @ np.asarray(bv, dtype=np.float32))   # [768]
    out = np.empty((B, T, C), dtype=np.float32)
    for b in range(B):
        acc = res.results[b * 4 + 0]["out"].astype(np.float32).copy()
        for g in range(1, 4):
            acc += res.results[b * 4 + g]["out"]
        out[b] = acc.T + const[None, :]
    return out


# revision 9
# speedup vs baseline: 2.0911x; 1.5056x over previous
"""Trainium2 Bass kernel for CausalCrossConditionalSelfAttention.

Reference semantics (B=2, T=2560, C=768, H=12, hd=64, t=T//10=256):
  q/k/v = x @ W{q,k,v}.T + b{q,k,v}           (per-head slices of C)
  att   = softmax(mask(q k^T / sqrt(hd)))      mask: (i%256) >= (j%256)
  y     = (att @ v) @ Wp.T + bp

Sharding: 8 cores = 2 batches x 4 head-groups (3 heads each).
Each core computes its (batch, 3 heads) slab fully on-chip and returns a
partial pre-projection output out^T [768, 2560]; the host sums the 4
head-group partials per batch and adds the constant bias (bp + Wp @ bv).

Device-side layout (per core):
  xT      [768, 2560]  x[b].T
  wqk     [768, 384]   cols: [Qh0|Qh1 | Kh0|Kh1 | Qh2 | Kh2] weight.T cols
  bqk     [4, 128, 1]  per-partition bias rows for the 4 col-groups
  wv      [768, 192]   Wv rows for the 3 heads, transposed
  wp      [3, 64, 768] per-head Wp[:, head_slice].T
  out     [768, 2560]  partial out^T (pre-bias)

The scores are computed transposed: S^T[k, q] in PSUM, exp'd on ScalarE
(scale=1/8 fused), masked by GPSIMD affine_select (exact zeros), and
contracted with V (ones column appended -> softmax denominator for free).
The (k%256)>=128 x (q%256)<128 quarter of each 256x256 mask block is fully
masked and skipped entirely (25% of score/AV/exp work).
"""

import numpy as np

B, T, C = 2, 2560, 768
H, HD = 12, 64
HPG = 3            # heads per group (core)
CW = HPG * HD      # 192
NKC = T // 128     # 20 key chunks of 128
NQT = T // 512     # 5 query tiles of 512
N_CORES = 8

_CACHE = {}


def _split_multi_waits(nc, maxw=1):
    """walrus in this container rejects >1 sync wait per instruction;
    split extra waits onto preceding NOPs on the same engine."""
    import concourse.mybir as mybir
    for f in nc.m.functions:
        for bb in f.blocks:
            newlist = []
            for ins in bb.instructions:
                si = ins.sync_info
                if si is not None and si.on_wait and len(si.on_wait) > maxw:
                    waits = list(si.on_wait)
                    chunks = [waits[i:i + maxw] for i in range(0, len(waits), maxw)]
                    for ch in chunks[:-1]:
                        newlist.append(mybir.InstNoOp(
                            name=f"WSPLIT-{nc.next_id()}",
                            engine=ins.engine,
                            sync_info=mybir.SyncInfo(on_wait=list(ch), on_update=[]),
                            text_hint="wait_split",
                        ))
                    ins.sync_info = mybir.SyncInfo(
                        on_wait=list(chunks[-1]), on_update=list(si.on_update))
                newlist.append(ins)
            bb.instructions = newlist
    return nc


def _chunks(lst, n):
    return [lst[i:i + n] for i in range(0, len(lst), n)]


def build_program():
    import concourse.bass as bass
    import concourse.mybir as mybir
    import concourse.tile as tile

    f32 = mybir.dt.float32
    bf16 = mybir.dt.bfloat16
    AF = mybir.ActivationFunctionType
    ALU = mybir.AluOpType

    nc = bass.Bass()
    xT = nc.dram_tensor("xT", [C, T], bf16, kind="ExternalInput")
    wqk = nc.dram_tensor("wqk", [C, 384], bf16, kind="ExternalInput")
    bqk = nc.dram_tensor("bqk", [4, 128, 1], f32, kind="ExternalInput")
    wv = nc.dram_tensor("wv", [C, CW], bf16, kind="ExternalInput")
    wp = nc.dram_tensor("wp", [HPG, HD, C], bf16, kind="ExternalInput")
    identm = nc.dram_tensor("identm", [128, 384], f32, kind="ExternalInput")
    out = nc.dram_tensor("out", [C, T], f32, kind="ExternalOutput")

    with tile.TileContext(nc) as tc:
        with tc.tile_pool(name="persist", bufs=1) as persist, \
             tc.tile_pool(name="work", bufs=2) as work, \
             tc.tile_pool(name="psum", bufs=2, space="PSUM") as psum:

            # ---------------- load inputs ----------------
            xt_sb = persist.tile([128, 6, T], bf16)       # x^T, 6 chunks of C
            for c in range(6):
                for qt in range(NQT):
                    nc.sync.dma_start(
                        out=xt_sb[:, c, qt * 512:(qt + 1) * 512],
                        in_=xT[c * 128:(c + 1) * 128, qt * 512:(qt + 1) * 512])
            wqk_sb = persist.tile([128, 6, 384], bf16)
            for c in range(6):
                nc.sync.dma_start(out=wqk_sb[:, c, :],
                                  in_=wqk[c * 128:(c + 1) * 128, :])
            wv_sb = persist.tile([128, 6, CW], bf16)
            for c in range(6):
                nc.sync.dma_start(out=wv_sb[:, c, :],
                                  in_=wv[c * 128:(c + 1) * 128, :])
            wp_sb = persist.tile([64, HPG, C], bf16)
            for h in range(HPG):
                nc.sync.dma_start(out=wp_sb[:, h, :], in_=wp[h])
            bqk_sb = persist.tile([128, 4, 1], f32)
            for j in range(4):
                nc.sync.dma_start(out=bqk_sb[:, j, :], in_=bqk[j])

            ones_sb = persist.tile([128, 64], f32)
            nc.vector.memset(ones_sb, 1.0)
            identm_f = work.tile([128, 384], f32, tag="im", bufs=1, name="identm_f")
            nc.sync.dma_start(out=identm_f, in_=identm[:, :])
            ident_sb = persist.tile([128, 128], bf16)   # identity
            maskm_sb = persist.tile([128, 256], bf16)   # [L|L], L=-1e9 if j<i
            nc.vector.tensor_copy(ident_sb, identm_f[:, 0:128])
            nc.vector.tensor_copy(maskm_sb, identm_f[:, 128:384])

            # ---------------- q/k projections (transposed) ----------------
            # qkT j0=[Qh0|Qh1] j1=[Kh0|Kh1] (128 parts); j2=Qh2 j3=Kh2 (64)
            qkT01q = persist.tile([128, T], bf16)
            qkT01k = persist.tile([128, T], bf16)
            qkT2q = persist.tile([64, T], bf16)
            qkT2k = persist.tile([64, T], bf16)
            jdefs = [(qkT01q, 0, 128), (qkT01k, 128, 128),
                     (qkT2q, 256, 64), (qkT2k, 320, 64)]
            for j, (dst, col0, m) in enumerate(jdefs):
                for qt in range(NQT):
                    qk_ps = psum.tile([128, 512], f32, tag="av", name="qk_ps")
                    for c in range(6):
                        nc.tensor.matmul(
                            qk_ps[:m, :],
                            lhsT=wqk_sb[:, c, col0:col0 + m],
                            rhs=xt_sb[:, c, qt * 512:(qt + 1) * 512],
                            start=(c == 0), stop=(c == 5))
                    nc.vector.tensor_scalar_add(
                        dst[:m, qt * 512:(qt + 1) * 512],
                        qk_ps[:m, :], bqk_sb[:m, j, :])

            # ---------------- v projection (natural layout + ones col) ----
            # per head h: cols [65h .. 65h+63] = V_h, col 65h+64 = 1.0
            v_sb = persist.tile([128, NKC, HPG * 65], bf16)
            v_r = v_sb.rearrange("p n (h c) -> p n h c", c=65)
            nc.vector.memset(v_r[:, :, :, 64], 1.0)
            for tch in range(NKC):
                v_ps = psum.tile([128, 512], f32, tag="av", name="v_ps")
                for c in range(6):
                    nc.tensor.matmul(
                        v_ps[:, :CW],
                        lhsT=xt_sb[:, c, tch * 128:(tch + 1) * 128],
                        rhs=wv_sb[:, c, :],
                        start=(c == 0), stop=(c == 5))
                nc.vector.tensor_copy(
                    v_r[:, tch, :, 0:64],
                    v_ps[:, :CW].rearrange("p (h c) -> p h c", h=HPG))

            # ---------------- attention ----------------
            evens = list(range(0, NKC, 2))
            odds = list(range(1, NKC, 2))
            ynorm = [persist.tile([64, T], bf16, name=f"ynorm{h}")
                     for h in range(HPG)]

            for h in range(HPG):
                if h < 2:
                    qTh = qkT01q[64 * h:64 * (h + 1), :]
                    kTh = qkT01k[64 * h:64 * (h + 1), :]
                else:
                    qTh = qkT2q[0:64, :]
                    kTh = qkT2k[0:64, :]
                # odd-subchunk view of q: [64, qt, two, sp, 128]
                q_odd = qTh.rearrange("p (q s t c) -> p q t s c",
                                      q=NQT, s=2, t=2, c=128)

                for qt in range(NQT):
                    qwin = qTh[:, qt * 512:(qt + 1) * 512]
                    av = psum.tile([128, 512], f32, tag="av", name="av")
                    av_odd = av.rearrange("p (s t c) -> p t s c",
                                          s=2, t=2, c=128)[:, 1]

                    for grp in _chunks(evens, 3):
                        L = len(grp)
                        sc = psum.tile([128, 1536], f32, tag="sc", name="sc")
                        for i, kc in enumerate(grp):
                            nc.tensor.matmul(
                                sc[:, i * 512:(i + 1) * 512],
                                lhsT=kTh[:, kc * 128:(kc + 1) * 128],
                                rhs=qwin, start=True, stop=False,
                                skip_group_check=True)
                        sc_r = sc.rearrange("p (l s t c) -> p l s t c",
                                            l=3, s=2, t=2, c=128)
                        for i in range(L):
                            nc.tensor.matmul(
                                sc_r[:, i, :, 0], lhsT=ident_sb, rhs=maskm_sb,
                                start=False, stop=True, skip_group_check=True)
                        pt = work.tile([128, 1536], bf16, tag="pt", name="pt")
                        nc.scalar.activation(pt[:, :L * 512], sc[:, :L * 512],
                                             AF.Exp, scale=0.125)
                        for i, kc in enumerate(grp):
                            nc.tensor.matmul(
                                av[:65, :],
                                lhsT=v_sb[:, kc, 65 * h:65 * h + 65],
                                rhs=pt[:, i * 512:(i + 1) * 512],
                                start=(kc == 0), stop=False,
                                skip_group_check=True)

                    for gi, grp in enumerate(_chunks(odds, 3)):
                        L = len(grp)
                        last_grp = (gi == 3)
                        sc = psum.tile([128, 1536], f32, tag="sc", name="sc")
                        for i, kc in enumerate(grp):
                            # 256-wide blocks: two share a 2KB psum bank, and
                            # start=True zero-marks the WHOLE bank - only the
                            # first block of each bank may set it.
                            nc.tensor.matmul(
                                sc[:, i * 256:(i + 1) * 256],
                                lhsT=kTh[:, kc * 128:(kc + 1) * 128],
                                rhs=q_odd[:, qt, 1], start=(i % 2 == 0),
                                stop=False, skip_group_check=True)
                        for i in range(L):
                            nc.tensor.matmul(
                                sc[:, i * 256:(i + 1) * 256],
                                lhsT=ident_sb, rhs=maskm_sb,
                                start=False, stop=True, skip_group_check=True)
                        pt = work.tile([128, 1536], bf16, tag="pt", name="pt")
                        nc.scalar.activation(pt[:, :L * 256], sc[:, :L * 256],
                                             AF.Exp, scale=0.125)
                        for i, kc in enumerate(grp):
                            nc.tensor.matmul(
                                av_odd[:65],
                                lhsT=v_sb[:, kc, 65 * h:65 * h + 65],
                                rhs=pt[:, i * 256:(i + 1) * 256],
                                start=False, stop=(kc == NKC - 1),
                                skip_group_check=True)

                    # normalize: y = av[0:64] / av[64]  (denominator row)
                    rcp = work.tile([65, 512], f32, tag="rcp", name="rcp")
                    nc.vector.reciprocal(rcp[64:65, :], av[64:65, :])
                    bc_ps = psum.tile([128, 512], f32, tag="av", name="bc_ps")
                    nc.tensor.matmul(bc_ps[:64, :],
                                     lhsT=ones_sb[64:65, :],
                                     rhs=rcp[64:65, :],
                                     start=True, stop=True)
                    bc_sb = work.tile([64, 512], f32, tag="bc", name="bc_sb")
                    nc.vector.tensor_copy(bc_sb, bc_ps[:64, :])
                    nc.vector.tensor_mul(
                        ynorm[h][:, qt * 512:(qt + 1) * 512],
                        av[0:64, :], bc_sb)

            # ---------------- output projection ----------------
            for m in range(6):
                for qt in range(NQT):
                    pj_ps = psum.tile([128, 512], f32, tag="av", name="pj_ps")
                    for h in range(HPG):
                        nc.tensor.matmul(
                            pj_ps,
                            lhsT=wp_sb[:, h, m * 128:(m + 1) * 128],
                            rhs=ynorm[h][:, qt * 512:(qt + 1) * 512],
                            start=(h == 0), stop=(h == 2))
                    pj_sb = work.tile([128, 512], f32, tag="pj", name="pj_sb")
                    nc.vector.tensor_copy(pj_sb, pj_ps)
                    nc.sync.dma_start(
                        out=out[m * 128:(m + 1) * 128, qt * 512:(qt + 1) * 512],
                        in_=pj_sb)

    _split_multi_waits(nc)
    return nc


def get_program():
    if "nc" not in _CACHE:
        _CACHE["nc"] = build_program()
    return _CACHE["nc"]


def make_in_maps(x, Wk, bk, Wq, bq, Wv, bv, Wp, bp):
    x = np.asarray(x, dtype=np.float32)
    in_maps = []
    for core in range(N_CORES):
        b, g = divmod(core, 4)
        h0 = g * HPG
        r = slice(h0 * HD, (h0 + HPG) * HD)     # 192 head dims
        xt = np.ascontiguousarray(x[b].T)
        wq_g = np.asarray(Wq)[r]                 # [192, 768]
        wk_g = np.asarray(Wk)[r]
        # wqk cols: [Qh0|Qh1(128) | Kh0|Kh1(128) | Qh2(64) | Kh2(64)]
        wqk = np.concatenate(
            [wq_g[:128].T, wk_g[:128].T, wq_g[128:].T, wk_g[128:].T],
            axis=1).astype(np.float32)
        bq_g = np.asarray(bq)[r].astype(np.float32)
        bk_g = np.asarray(bk)[r].astype(np.float32)
        bqk = np.zeros((4, 128, 1), np.float32)
        bqk[0, :, 0] = bq_g[:128]
        bqk[1, :, 0] = bk_g[:128]
        bqk[2, :64, 0] = bq_g[128:]
        bqk[3, :64, 0] = bk_g[128:]
        wv_g = np.ascontiguousarray(np.asarray(Wv)[r].T).astype(np.float32)
        wp_g = np.asarray(Wp)[:, r]              # [768, 192]
        wp_t = np.ascontiguousarray(
            wp_g.T.reshape(HPG, HD, C)).astype(np.float32)
        ident = np.eye(128, dtype=np.float32)
        L = np.where(np.arange(256)[None, :] % 128 < np.arange(128)[:, None],
                     np.float32(-1e9), np.float32(0.0))
        identm = np.concatenate([ident, L], axis=1).astype(np.float32)
        import ml_dtypes
        b16 = ml_dtypes.bfloat16
        in_maps.append({
            "identm": identm,
            "xT": np.ascontiguousarray(xt).astype(b16),
            "wqk": np.ascontiguousarray(wqk).astype(b16),
            "bqk": bqk,
            "wv": wv_g.astype(b16),
            "wp": wp_t.astype(b16),
        })
    return in_maps


def kernel(x, Wk, bk, Wq, bq, Wv, bv, Wp, bp):
    from concourse.bass_utils import run_bass_kernel_spmd
    nc = get_program()
    in_maps = make_in_maps(x, Wk, bk, Wq, bq, Wv, bv, Wp, bp)
    res = run_bass_kernel_spmd(nc, in_maps, list(range(N_CORES)))
    Wp_np = np.asarray(Wp, dtype=np.float32)
    const = (np.asarray(bp, dtype=np.float32)
             + Wp_np @ np.asarray(bv, dtype=np.float32))   # [768]
    out = np.empty((B, T, C), dtype=np.float32)
    for b in range(B):
        acc = res.results[b * 4 + 0]["out"].astype(np.float32).copy()
        for g in range(1, 4):
            acc += res.results[b * 4 + g]["out"]
        out[b] = acc.T + const[None, :]
    return out


# revision 13
# speedup vs baseline: 2.3456x; 1.1217x over previous
"""Trainium2 Bass kernel for CausalCrossConditionalSelfAttention.

Reference semantics (B=2, T=2560, C=768, H=12, hd=64, t=T//10=256):
  q/k/v = x @ W{q,k,v}.T + b{q,k,v}           (per-head slices of C)
  att   = softmax(mask(q k^T / sqrt(hd)))      mask: (i%256) >= (j%256)
  y     = (att @ v) @ Wp.T + bp

Sharding: 8 cores = 2 batches x 4 head-groups (3 heads each).
Each core computes its (batch, 3 heads) slab fully on-chip and returns a
partial pre-projection output out^T [768, 2560]; the host sums the 4
head-group partials per batch and adds the constant bias (bp + Wp @ bv).

Device-side layout (per core):
  xT      [768, 2560]  x[b].T
  wqk     [768, 384]   cols: [Qh0|Qh1 | Kh0|Kh1 | Qh2 | Kh2] weight.T cols
  bqk     [4, 128, 1]  per-partition bias rows for the 4 col-groups
  wv      [768, 192]   Wv rows for the 3 heads, transposed
  wp      [3, 64, 768] per-head Wp[:, head_slice].T
  out     [768, 2560]  partial out^T (pre-bias)

The scores are computed transposed: S^T[k, q] in PSUM, exp'd on ScalarE
(scale=1/8 fused), masked by GPSIMD affine_select (exact zeros), and
contracted with V (ones column appended -> softmax denominator for free).
The (k%256)>=128 x (q%256)<128 quarter of each 256x256 mask block is fully
masked and skipped entirely (25% of score/AV/exp work).
"""

import numpy as np

B, T, C = 2, 2560, 768
H, HD = 12, 64
HPG = 3            # heads per group (core)
CW = HPG * HD      # 192
NKC = T // 128     # 20 key chunks of 128
NQT = T // 512     # 5 query tiles of 512
N_CORES = 8

_CACHE = {}


def _split_multi_waits(nc, maxw=1):
    """walrus in this container rejects >1 sync wait per instruction;
    split extra waits onto preceding NOPs on the same engine."""
    import concourse.mybir as mybir
    for f in nc.m.functions:
        for bb in f.blocks:
            newlist = []
            for ins in bb.instructions:
                si = ins.sync_info
                if si is not None and si.on_wait and len(si.on_wait) > maxw:
                    waits = list(si.on_wait)
                    chunks = [waits[i:i + maxw] for i in range(0, len(waits), maxw)]
                    for ch in chunks[:-1]:
                        newlist.append(mybir.InstNoOp(
                            name=f"WSPLIT-{nc.next_id()}",
                            engine=ins.engine,
                            sync_info=mybir.SyncInfo(on_wait=list(ch), on_update=[]),
                            text_hint="wait_split",
                        ))
                    ins.sync_info = mybir.SyncInfo(
                        on_wait=list(chunks[-1]), on_update=list(si.on_update))
                newlist.append(ins)
            bb.instructions = newlist
    return nc


def _chunks(lst, n):
    return [lst[i:i + n] for i in range(0, len(lst), n)]


def build_program():
    import concourse.bass as bass
    import concourse.mybir as mybir
    import concourse.tile as tile

    f32 = mybir.dt.float32
    bf16 = mybir.dt.bfloat16
    AF = mybir.ActivationFunctionType
    ALU = mybir.AluOpType

    nc = bass.Bass()
    xT = nc.dram_tensor("xT", [C, T], bf16, kind="ExternalInput")
    wqk = nc.dram_tensor("wqk", [C, 384], bf16, kind="ExternalInput")
    bqk = nc.dram_tensor("bqk", [4, 128, 1], f32, kind="ExternalInput")
    wv = nc.dram_tensor("wv", [C, CW], bf16, kind="ExternalInput")
    wp = nc.dram_tensor("wp", [HPG, HD, C], bf16, kind="ExternalInput")
    identm = nc.dram_tensor("identm", [128, 384], f32, kind="ExternalInput")
    out = nc.dram_tensor("out", [C, T], f32, kind="ExternalOutput")

    with tile.TileContext(nc) as tc:
        with tc.tile_pool(name="persist", bufs=1) as persist, \
             tc.tile_pool(name="work", bufs=2) as work, \
             tc.tile_pool(name="psum", bufs=2, space="PSUM") as psum:

            # ---------------- load inputs ----------------
            xt_sb = persist.tile([128, 6, T], bf16)       # x^T, 6 chunks of C
            for qt in range(NQT):
                for c in range(6):
                    nc.sync.dma_start(
                        out=xt_sb[:, c, qt * 512:(qt + 1) * 512],
                        in_=xT[c * 128:(c + 1) * 128, qt * 512:(qt + 1) * 512])
            wqk_sb = persist.tile([128, 6, 384], bf16)
            for c in range(6):
                nc.sync.dma_start(out=wqk_sb[:, c, :],
                                  in_=wqk[c * 128:(c + 1) * 128, :])
            wv_sb = persist.tile([128, 6, CW], bf16)
            for c in range(6):
                nc.sync.dma_start(out=wv_sb[:, c, :],
                                  in_=wv[c * 128:(c + 1) * 128, :])
            wp_sb = persist.tile([64, HPG, C], bf16)
            for h in range(HPG):
                nc.sync.dma_start(out=wp_sb[:, h, :], in_=wp[h])
            bqk_sb = persist.tile([128, 4, 1], f32)
            for j in range(4):
                nc.sync.dma_start(out=bqk_sb[:, j, :], in_=bqk[j])

            ones_sb = persist.tile([128, 64], f32)
            nc.vector.memset(ones_sb, 1.0)
            identm_f = work.tile([128, 384], f32, tag="im", bufs=1, name="identm_f")
            nc.sync.dma_start(out=identm_f, in_=identm[:, :])
            ident_sb = persist.tile([128, 128], bf16)   # identity
            maskm_sb = persist.tile([128, 256], bf16)   # [L|L], L=-1e9 if j<i
            nc.vector.tensor_copy(ident_sb, identm_f[:, 0:128])
            nc.vector.tensor_copy(maskm_sb, identm_f[:, 128:384])

            # ---------------- q/k projections (transposed) ----------------
            # qkT j0=[Qh0|Qh1] j1=[Kh0|Kh1] (128 parts); j2=Qh2 j3=Kh2 (64)
            qkT01q = persist.tile([128, T], bf16)
            qkT01k = persist.tile([128, T], bf16)
            qkT2q = persist.tile([64, T], bf16)
            qkT2k = persist.tile([64, T], bf16)
            jdefs = [(qkT01q, 0, 128), (qkT01k, 128, 128),
                     (qkT2q, 256, 64), (qkT2k, 320, 64)]
            for j, (dst, col0, m) in enumerate(jdefs):
                for qt in range(NQT):
                    qk_ps = psum.tile([128, 512], f32, tag="av", name="qk_ps")
                    for c in range(6):
                        nc.tensor.matmul(
                            qk_ps[:m, :],
                            lhsT=wqk_sb[:, c, col0:col0 + m],
                            rhs=xt_sb[:, c, qt * 512:(qt + 1) * 512],
                            start=(c == 0), stop=(c == 5))
                    nc.vector.tensor_scalar_add(
                        dst[:m, qt * 512:(qt + 1) * 512],
                        qk_ps[:m, :], bqk_sb[:m, j, :])

            # ---------------- v projection (natural layout + ones col) ----
            # per head h: cols [65h .. 65h+63] = V_h, col 65h+64 = 1.0
            v_sb = persist.tile([128, NKC, HPG * 65], bf16)
            v_r = v_sb.rearrange("p n (h c) -> p n h c", c=65)
            nc.vector.memset(v_r[:, :, :, 64], 1.0)
            for tch in range(NKC):
                v_ps = psum.tile([128, 512], f32, tag="av", name="v_ps")
                for c in range(6):
                    nc.tensor.matmul(
                        v_ps[:, :CW],
                        lhsT=xt_sb[:, c, tch * 128:(tch + 1) * 128],
                        rhs=wv_sb[:, c, :],
                        start=(c == 0), stop=(c == 5))
                nc.vector.tensor_copy(
                    v_r[:, tch, :, 0:64],
                    v_ps[:, :CW].rearrange("p (h c) -> p h c", h=HPG))

            # ---------------- attention ----------------
            evens = list(range(0, NKC, 2))
            odds = list(range(1, NKC, 2))
            ynorm = [persist.tile([64, T], bf16, name=f"ynorm{h}")
                     for h in range(HPG)]

            pending = []

            def _flush_norm(item):
                av_p, rcp_p, h_p, qt_p = item
                bc_ps = psum.tile([128, 512], f32, tag="sc", name="bc_ps")
                nc.tensor.matmul(bc_ps[:64, :],
                                 lhsT=ones_sb[64:65, :],
                                 rhs=rcp_p[64:65, :],
                                 start=True, stop=True)
                bc_sb = work.tile([64, 512], f32, tag="bc", name="bc_sb")
                nc.vector.tensor_copy(bc_sb, bc_ps[:64, :])
                nc.vector.tensor_mul(
                    ynorm[h_p][:, qt_p * 512:(qt_p + 1) * 512],
                    av_p[0:64, :], bc_sb)

            for h in range(HPG):
                if h < 2:
                    qTh = qkT01q[64 * h:64 * (h + 1), :]
                    kTh = qkT01k[64 * h:64 * (h + 1), :]
                else:
                    qTh = qkT2q[0:64, :]
                    kTh = qkT2k[0:64, :]
                # odd-subchunk view of q: [64, qt, two, sp, 128]
                q_odd = qTh.rearrange("p (q s t c) -> p q t s c",
                                      q=NQT, s=2, t=2, c=128)

                for qt in range(NQT):
                    qwin = qTh[:, qt * 512:(qt + 1) * 512]
                    av = psum.tile([128, 512], f32, tag="av", name="av")
                    av_odd = av.rearrange("p (s t c) -> p t s c",
                                          s=2, t=2, c=128)[:, 1]

                    for grp in _chunks(evens, 3):
                        L = len(grp)
                        sc = psum.tile([128, 1536], f32, tag="sc", name="sc")
                        for i, kc in enumerate(grp):
                            nc.tensor.matmul(
                                sc[:, i * 512:(i + 1) * 512],
                                lhsT=kTh[:, kc * 128:(kc + 1) * 128],
                                rhs=qwin, start=True, stop=False,
                                skip_group_check=True)
                        sc_r = sc.rearrange("p (l s t c) -> p l s t c",
                                            l=3, s=2, t=2, c=128)
                        for i in range(L):
                            nc.tensor.matmul(
                                sc_r[:, i, :, 0], lhsT=ident_sb, rhs=maskm_sb,
                                start=False, stop=True, skip_group_check=True)
                        pt = work.tile([128, 1536], bf16, tag="pt", name="pt")
                        nc.scalar.activation(pt[:, :L * 512], sc[:, :L * 512],
                                             AF.Exp, scale=0.125)
                        for i, kc in enumerate(grp):
                            nc.tensor.matmul(
                                av[:65, :],
                                lhsT=v_sb[:, kc, 65 * h:65 * h + 65],
                                rhs=pt[:, i * 512:(i + 1) * 512],
                                start=(kc == 0), stop=False,
                                skip_group_check=True)

                    if pending:
                        _flush_norm(pending.pop(0))
                    for gi, grp in enumerate(_chunks(odds, 3)):
                        L = len(grp)
                        last_grp = (gi == 3)
                        sc = psum.tile([128, 1536], f32, tag="sc", name="sc")
                        for i, kc in enumerate(grp):
                            # 256-wide blocks: two share a 2KB psum bank, and
                            # start=True zero-marks the WHOLE bank - only the
                            # first block of each bank may set it.
                            nc.tensor.matmul(
                                sc[:, i * 256:(i + 1) * 256],
                                lhsT=kTh[:, kc * 128:(kc + 1) * 128],
                                rhs=q_odd[:, qt, 1], start=(i % 2 == 0),
                                stop=False, skip_group_check=True)
                        for i in range(L):
                            nc.tensor.matmul(
                                sc[:, i * 256:(i + 1) * 256],
                                lhsT=ident_sb, rhs=maskm_sb,
                                start=False, stop=True, skip_group_check=True)
                        pt = work.tile([128, 1536], bf16, tag="pt", name="pt")
                        nc.scalar.activation(pt[:, :L * 256], sc[:, :L * 256],
                                             AF.Exp, scale=0.125)
                        for i, kc in enumerate(grp):
                            nc.tensor.matmul(
                                av_odd[:65],
                                lhsT=v_sb[:, kc, 65 * h:65 * h + 65],
                                rhs=pt[:, i * 256:(i + 1) * 256],
                                start=False, stop=(kc == NKC - 1),
                                skip_group_check=True)

                    # normalize: y = av[0:64] / av[64]  (denominator row)
                    rcp = work.tile([65, 512], f32, tag="rcp", name="rcp",
                                    bufs=3)
                    nc.vector.reciprocal(rcp[64:65, :], av[64:65, :])
                    pending.append((av, rcp, h, qt))

            while pending:
                _flush_norm(pending.pop(0))

            # ---------------- output projection ----------------
            for m in range(6):
                for qt in range(NQT):
                    pj_ps = psum.tile([128, 512], f32, tag="av", name="pj_ps")
                    for h in range(HPG):
                        nc.tensor.matmul(
                            pj_ps,
                            lhsT=wp_sb[:, h, m * 128:(m + 1) * 128],
                            rhs=ynorm[h][:, qt * 512:(qt + 1) * 512],
                            start=(h == 0), stop=(h == 2))
                    pj_sb = work.tile([128, 512], f32, tag="pj", name="pj_sb")
                    nc.vector.tensor_copy(pj_sb, pj_ps)
                    nc.sync.dma_start(
                        out=out[m * 128:(m + 1) * 128, qt * 512:(qt + 1) * 512],
                        in_=pj_sb)

    _split_multi_waits(nc)
    return nc


def get_program():
    if "nc" not in _CACHE:
        _CACHE["nc"] = build_program()
    return _CACHE["nc"]


def make_in_maps(x, Wk, bk, Wq, bq, Wv, bv, Wp, bp):
    x = np.asarray(x, dtype=np.float32)
    in_maps = []
    for core in range(N_CORES):
        b, g = divmod(core, 4)
        h0 = g * HPG
        r = slice(h0 * HD, (h0 + HPG) * HD)     # 192 head dims
        xt = np.ascontiguousarray(x[b].T)
        wq_g = np.asarray(Wq)[r]                 # [192, 768]
        wk_g = np.asarray(Wk)[r]
        # wqk cols: [Qh0|Qh1(128) | Kh0|Kh1(128) | Qh2(64) | Kh2(64)]
        wqk = np.concatenate(
            [wq_g[:128].T, wk_g[:128].T, wq_g[128:].T, wk_g[128:].T],
            axis=1).astype(np.float32)
        bq_g = np.asarray(bq)[r].astype(np.float32)
        bk_g = np.asarray(bk)[r].astype(np.float32)
        bqk = np.zeros((4, 128, 1), np.float32)
        bqk[0, :, 0] = bq_g[:128]
        bqk[1, :, 0] = bk_g[:128]
        bqk[2, :64, 0] = bq_g[128:]
        bqk[3, :64, 0] = bk_g[128:]
        wv_g = np.ascontiguousarray(np.asarray(Wv)[r].T).astype(np.float32)
        wp_g = np.asarray(Wp)[:, r]              # [768, 192]
        wp_t = np.ascontiguousarray(
            wp_g.T.reshape(HPG, HD, C)).astype(np.float32)
        ident = np.eye(128, dtype=np.float32)
        L = np.where(np.arange(256)[None, :] % 128 < np.arange(128)[:, None],
                     np.float32(-1e9), np.float32(0.0))
        identm = np.concatenate([ident, L], axis=1).astype(np.float32)
        import ml_dtypes
        b16 = ml_dtypes.bfloat16
        in_maps.append({
            "identm": identm,
            "xT": np.ascontiguousarray(xt).astype(b16),
            "wqk": np.ascontiguousarray(wqk).astype(b16),
            "bqk": bqk,
            "wv": wv_g.astype(b16),
            "wp": wp_t.astype(b16),
        })
    return in_maps


def kernel(x, Wk, bk, Wq, bq, Wv, bv, Wp, bp):
    from concourse.bass_utils import run_bass_kernel_spmd
    nc = get_program()
    in_maps = make_in_maps(x, Wk, bk, Wq, bq, Wv, bv, Wp, bp)
    res = run_bass_kernel_spmd(nc, in_maps, list(range(N_CORES)))
    Wp_np = np.asarray(Wp, dtype=np.float32)
    const = (np.asarray(bp, dtype=np.float32)
             + Wp_np @ np.asarray(bv, dtype=np.float32))   # [768]
    out = np.empty((B, T, C), dtype=np.float32)
    for b in range(B):
        acc = res.results[b * 4 + 0]["out"].astype(np.float32).copy()
        for g in range(1, 4):
            acc += res.results[b * 4 + g]["out"]
        out[b] = acc.T + const[None, :]
    return out


# revision 14
# speedup vs baseline: 2.3768x; 1.0133x over previous
"""Trainium2 Bass kernel for CausalCrossConditionalSelfAttention.

Reference semantics (B=2, T=2560, C=768, H=12, hd=64, t=T//10=256):
  q/k/v = x @ W{q,k,v}.T + b{q,k,v}           (per-head slices of C)
  att   = softmax(mask(q k^T / sqrt(hd)))      mask: (i%256) >= (j%256)
  y     = (att @ v) @ Wp.T + bp

Sharding: 8 cores = 2 batches x 4 head-groups (3 heads each).
Each core computes its (batch, 3 heads) slab fully on-chip and returns a
partial pre-projection output out^T [768, 2560]; the host sums the 4
head-group partials per batch and adds the constant bias (bp + Wp @ bv).

Device-side layout (per core):
  xT      [768, 2560]  x[b].T
  wqk     [768, 384]   cols: [Qh0|Qh1 | Kh0|Kh1 | Qh2 | Kh2] weight.T cols
  bqk     [4, 128, 1]  per-partition bias rows for the 4 col-groups
  wv      [768, 192]   Wv rows for the 3 heads, transposed
  wp      [3, 64, 768] per-head Wp[:, head_slice].T
  out     [768, 2560]  partial out^T (pre-bias)

The scores are computed transposed: S^T[k, q] in PSUM, exp'd on ScalarE
(scale=1/8 fused), masked by GPSIMD affine_select (exact zeros), and
contracted with V (ones column appended -> softmax denominator for free).
The (k%256)>=128 x (q%256)<128 quarter of each 256x256 mask block is fully
masked and skipped entirely (25% of score/AV/exp work).
"""

import numpy as np

B, T, C = 2, 2560, 768
H, HD = 12, 64
HPG = 3            # heads per group (core)
CW = HPG * HD      # 192
NKC = T // 128     # 20 key chunks of 128
NQT = T // 512     # 5 query tiles of 512
N_CORES = 8

_CACHE = {}


def _split_multi_waits(nc, maxw=1):
    """walrus in this container rejects >1 sync wait per instruction;
    split extra waits onto preceding NOPs on the same engine."""
    import concourse.mybir as mybir
    for f in nc.m.functions:
        for bb in f.blocks:
            newlist = []
            for ins in bb.instructions:
                si = ins.sync_info
                if si is not None and si.on_wait and len(si.on_wait) > maxw:
                    waits = list(si.on_wait)
                    chunks = [waits[i:i + maxw] for i in range(0, len(waits), maxw)]
                    for ch in chunks[:-1]:
                        newlist.append(mybir.InstNoOp(
                            name=f"WSPLIT-{nc.next_id()}",
                            engine=ins.engine,
                            sync_info=mybir.SyncInfo(on_wait=list(ch), on_update=[]),
                            text_hint="wait_split",
                        ))
                    ins.sync_info = mybir.SyncInfo(
                        on_wait=list(chunks[-1]), on_update=list(si.on_update))
                newlist.append(ins)
            bb.instructions = newlist
    return nc


def _chunks(lst, n):
    return [lst[i:i + n] for i in range(0, len(lst), n)]


def build_program():
    import concourse.bass as bass
    import concourse.mybir as mybir
    import concourse.tile as tile

    f32 = mybir.dt.float32
    bf16 = mybir.dt.bfloat16
    AF = mybir.ActivationFunctionType
    ALU = mybir.AluOpType

    nc = bass.Bass()
    xT = nc.dram_tensor("xT", [C, T], bf16, kind="ExternalInput")
    wqk = nc.dram_tensor("wqk", [C, 384], bf16, kind="ExternalInput")
    bqk = nc.dram_tensor("bqk", [4, 128, 1], f32, kind="ExternalInput")
    wv = nc.dram_tensor("wv", [C, CW], bf16, kind="ExternalInput")
    wp = nc.dram_tensor("wp", [HPG, HD, C], bf16, kind="ExternalInput")
    identm = nc.dram_tensor("identm", [128, 384], f32, kind="ExternalInput")
    out = nc.dram_tensor("out", [C, T], f32, kind="ExternalOutput")

    with tile.TileContext(nc) as tc:
        with tc.tile_pool(name="persist", bufs=1) as persist, \
             tc.tile_pool(name="work", bufs=2) as work, \
             tc.tile_pool(name="psum", bufs=2, space="PSUM") as psum:

            # ---------------- load inputs ----------------
            wqk_sb = persist.tile([128, 6, 384], bf16)
            for c in range(6):
                nc.sync.dma_start(out=wqk_sb[:, c, :],
                                  in_=wqk[c * 128:(c + 1) * 128, :])
            wv_sb = persist.tile([128, 6, CW], bf16)
            for c in range(6):
                nc.sync.dma_start(out=wv_sb[:, c, :],
                                  in_=wv[c * 128:(c + 1) * 128, :])
            wp_sb = persist.tile([64, HPG, C], bf16)
            for h in range(HPG):
                nc.sync.dma_start(out=wp_sb[:, h, :], in_=wp[h])
            bqk_sb = persist.tile([128, 4, 1], f32)
            for j in range(4):
                nc.sync.dma_start(out=bqk_sb[:, j, :], in_=bqk[j])
            xt_sb = persist.tile([128, 6, T], bf16)       # x^T, 6 chunks of C
            for qt in range(NQT):
                for c in range(6):
                    nc.sync.dma_start(
                        out=xt_sb[:, c, qt * 512:(qt + 1) * 512],
                        in_=xT[c * 128:(c + 1) * 128, qt * 512:(qt + 1) * 512])

            ones_sb = persist.tile([128, 64], f32)
            nc.vector.memset(ones_sb, 1.0)
            identm_f = work.tile([128, 384], f32, tag="im", bufs=1, name="identm_f")
            nc.sync.dma_start(out=identm_f, in_=identm[:, :])
            ident_sb = persist.tile([128, 128], bf16)   # identity
            maskm_sb = persist.tile([128, 256], bf16)   # [L|L], L=-1e9 if j<i
            nc.vector.tensor_copy(ident_sb, identm_f[:, 0:128])
            nc.vector.tensor_copy(maskm_sb, identm_f[:, 128:384])

            # ---------------- q/k projections (transposed) ----------------
            # qkT j0=[Qh0|Qh1] j1=[Kh0|Kh1] (128 parts); j2=Qh2 j3=Kh2 (64)
            qkT01q = persist.tile([128, T], bf16)
            qkT01k = persist.tile([128, T], bf16)
            qkT2q = persist.tile([64, T], bf16)
            qkT2k = persist.tile([64, T], bf16)
            jdefs = [(qkT01q, 0, 128), (qkT01k, 128, 128),
                     (qkT2q, 256, 64), (qkT2k, 320, 64)]
            for j, (dst, col0, m) in enumerate(jdefs):
                for qt in range(NQT):
                    qk_ps = psum.tile([128, 512], f32, tag="av", name="qk_ps")
                    for c in range(6):
                        nc.tensor.matmul(
                            qk_ps[:m, :],
                            lhsT=wqk_sb[:, c, col0:col0 + m],
                            rhs=xt_sb[:, c, qt * 512:(qt + 1) * 512],
                            start=(c == 0), stop=(c == 5))
                    nc.vector.tensor_scalar_add(
                        dst[:m, qt * 512:(qt + 1) * 512],
                        qk_ps[:m, :], bqk_sb[:m, j, :])

            # ---------------- v projection (natural layout + ones col) ----
            # per head h: cols [65h .. 65h+63] = V_h, col 65h+64 = 1.0
            v_sb = persist.tile([128, NKC, HPG * 65], bf16)
            v_r = v_sb.rearrange("p n (h c) -> p n h c", c=65)
            nc.vector.memset(v_r[:, :, :, 64], 1.0)
            for tch in range(NKC):
                v_ps = psum.tile([128, 512], f32, tag="av", name="v_ps")
                for c in range(6):
                    nc.tensor.matmul(
                        v_ps[:, :CW],
                        lhsT=xt_sb[:, c, tch * 128:(tch + 1) * 128],
                        rhs=wv_sb[:, c, :],
                        start=(c == 0), stop=(c == 5))
                nc.vector.tensor_copy(
                    v_r[:, tch, :, 0:64],
                    v_ps[:, :CW].rearrange("p (h c) -> p h c", h=HPG))

            # ---------------- attention ----------------
            evens = list(range(0, NKC, 2))
            odds = list(range(1, NKC, 2))
            ynorm = [persist.tile([64, T], bf16, name=f"ynorm{h}")
                     for h in range(HPG)]

            pending = []

            def _emit_proj(qt_p):
                for m in range(6):
                    pj_ps = psum.tile([128, 512], f32, tag="sc", name="pj_ps")
                    for hh in range(HPG):
                        nc.tensor.matmul(
                            pj_ps,
                            lhsT=wp_sb[:, hh, m * 128:(m + 1) * 128],
                            rhs=ynorm[hh][:, qt_p * 512:(qt_p + 1) * 512],
                            start=(hh == 0), stop=(hh == 2))
                    pj_sb = work.tile([128, 512], f32, tag="pj", name="pj_sb")
                    nc.vector.tensor_copy(pj_sb, pj_ps)
                    nc.sync.dma_start(
                        out=out[m * 128:(m + 1) * 128,
                                qt_p * 512:(qt_p + 1) * 512],
                        in_=pj_sb)

            def _flush_norm(item):
                av_p, rcp_p, h_p, qt_p = item
                bc_ps = psum.tile([128, 512], f32, tag="sc", name="bc_ps")
                nc.tensor.matmul(bc_ps[:64, :],
                                 lhsT=ones_sb[64:65, :],
                                 rhs=rcp_p[64:65, :],
                                 start=True, stop=True)
                bc_sb = work.tile([64, 512], f32, tag="bc", name="bc_sb")
                nc.vector.tensor_copy(bc_sb, bc_ps[:64, :])
                nc.vector.tensor_mul(
                    ynorm[h_p][:, qt_p * 512:(qt_p + 1) * 512],
                    av_p[0:64, :], bc_sb)
                if h_p == HPG - 1:
                    _emit_proj(qt_p)

            for qt in range(NQT):
                for h in range(HPG):
                    if h < 2:
                        qTh = qkT01q[64 * h:64 * (h + 1), :]
                        kTh = qkT01k[64 * h:64 * (h + 1), :]
                    else:
                        qTh = qkT2q[0:64, :]
                        kTh = qkT2k[0:64, :]
                    # odd-subchunk view of q: [64, qt, two, sp, 128]
                    q_odd = qTh.rearrange("p (q s t c) -> p q t s c",
                                          q=NQT, s=2, t=2, c=128)
                    qwin = qTh[:, qt * 512:(qt + 1) * 512]
                    av = psum.tile([128, 512], f32, tag="av", name="av")
                    av_odd = av.rearrange("p (s t c) -> p t s c",
                                          s=2, t=2, c=128)[:, 1]

                    for grp in _chunks(evens, 3):
                        L = len(grp)
                        sc = psum.tile([128, 1536], f32, tag="sc", name="sc")
                        for i, kc in enumerate(grp):
                            nc.tensor.matmul(
                                sc[:, i * 512:(i + 1) * 512],
                                lhsT=kTh[:, kc * 128:(kc + 1) * 128],
                                rhs=qwin, start=True, stop=False,
                                skip_group_check=True)
                        sc_r = sc.rearrange("p (l s t c) -> p l s t c",
                                            l=3, s=2, t=2, c=128)
                        for i in range(L):
                            nc.tensor.matmul(
                                sc_r[:, i, :, 0], lhsT=ident_sb, rhs=maskm_sb,
                                start=False, stop=True, skip_group_check=True)
                        pt = work.tile([128, 1536], bf16, tag="pt", name="pt")
                        nc.scalar.activation(pt[:, :L * 512], sc[:, :L * 512],
                                             AF.Exp, scale=0.125)
                        for i, kc in enumerate(grp):
                            nc.tensor.matmul(
                                av[:65, :],
                                lhsT=v_sb[:, kc, 65 * h:65 * h + 65],
                                rhs=pt[:, i * 512:(i + 1) * 512],
                                start=(kc == 0), stop=False,
                                skip_group_check=True)

                    if pending:
                        _flush_norm(pending.pop(0))
                    for gi, grp in enumerate(_chunks(odds, 3)):
                        L = len(grp)
                        last_grp = (gi == 3)
                        sc = psum.tile([128, 1536], f32, tag="sc", name="sc")
                        for i, kc in enumerate(grp):
                            # 256-wide blocks: two share a 2KB psum bank, and
                            # start=True zero-marks the WHOLE bank - only the
                            # first block of each bank may set it.
                            nc.tensor.matmul(
                                sc[:, i * 256:(i + 1) * 256],
                                lhsT=kTh[:, kc * 128:(kc + 1) * 128],
                                rhs=q_odd[:, qt, 1], start=(i % 2 == 0),
                                stop=False, skip_group_check=True)
                        for i in range(L):
                            nc.tensor.matmul(
                                sc[:, i * 256:(i + 1) * 256],
                                lhsT=ident_sb, rhs=maskm_sb,
                                start=False, stop=True, skip_group_check=True)
                        pt = work.tile([128, 1536], bf16, tag="pt", name="pt")
                        nc.scalar.activation(pt[:, :L * 256], sc[:, :L * 256],
                                             AF.Exp, scale=0.125)
                        for i, kc in enumerate(grp):
                            nc.tensor.matmul(
                                av_odd[:65],
                                lhsT=v_sb[:, kc, 65 * h:65 * h + 65],
                                rhs=pt[:, i * 256:(i + 1) * 256],
                                start=False, stop=(kc == NKC - 1),
                                skip_group_check=True)

                    # normalize: y = av[0:64] / av[64]  (denominator row)
                    rcp = work.tile([65, 512], f32, tag="rcp", name="rcp",
                                    bufs=3)
                    nc.vector.reciprocal(rcp[64:65, :], av[64:65, :])
                    pending.append((av, rcp, h, qt))

            while pending:
                _flush_norm(pending.pop(0))

    _split_multi_waits(nc)
    return nc


def get_program():
    if "nc" not in _CACHE:
        _CACHE["nc"] = build_program()
    return _CACHE["nc"]


def make_in_maps(x, Wk, bk, Wq, bq, Wv, bv, Wp, bp):
    x = np.asarray(x, dtype=np.float32)
    in_maps = []
    for core in range(N_CORES):
        b, g = divmod(core, 4)
        h0 = g * HPG
        r = slice(h0 * HD, (h0 + HPG) * HD)     # 192 head dims
        xt = np.ascontiguousarray(x[b].T)
        wq_g = np.asarray(Wq)[r]                 # [192, 768]
        wk_g = np.asarray(Wk)[r]
        # wqk cols: [Qh0|Qh1(128) | Kh0|Kh1(128) | Qh2(64) | Kh2(64)]
        wqk = np.concatenate(
            [wq_g[:128].T, wk_g[:128].T, wq_g[128:].T, wk_g[128:].T],
            axis=1).astype(np.float32)
        bq_g = np.asarray(bq)[r].astype(np.float32)
        bk_g = np.asarray(bk)[r].astype(np.float32)
        bqk = np.zeros((4, 128, 1), np.float32)
        bqk[0, :, 0] = bq_g[:128]
        bqk[1, :, 0] = bk_g[:128]
        bqk[2, :64, 0] = bq_g[128:]
        bqk[3, :64, 0] = bk_g[128:]
        wv_g = np.ascontiguousarray(np.asarray(Wv)[r].T).astype(np.float32)
        wp_g = np.asarray(Wp)[:, r]              # [768, 192]
        wp_t = np.ascontiguousarray(
            wp_g.T.reshape(HPG, HD, C)).astype(np.float32)
        ident = np.eye(128, dtype=np.float32)
        L = np.where(np.arange(256)[None, :] % 128 < np.arange(128)[:, None],
                     np.float32(-1e9), np.float32(0.0))
        identm = np.concatenate([ident, L], axis=1).astype(np.float32)
        import ml_dtypes
        b16 = ml_dtypes.bfloat16
        in_maps.append({
            "identm": identm,
            "xT": np.ascontiguousarray(xt).astype(b16),
            "wqk": np.ascontiguousarray(wqk).astype(b16),
            "bqk": bqk,
            "wv": wv_g.astype(b16),
            "wp": wp_t.astype(b16),
        })
    return in_maps


def kernel(x, Wk, bk, Wq, bq, Wv, bv, Wp, bp):
    from concourse.bass_utils import run_bass_kernel_spmd
    nc = get_program()
    in_maps = make_in_maps(x, Wk, bk, Wq, bq, Wv, bv, Wp, bp)
    res = run_bass_kernel_spmd(nc, in_maps, list(range(N_CORES)))
    Wp_np = np.asarray(Wp, dtype=np.float32)
    const = (np.asarray(bp, dtype=np.float32)
             + Wp_np @ np.asarray(bv, dtype=np.float32))   # [768]
    out = np.empty((B, T, C), dtype=np.float32)
    for b in range(B):
        acc = res.results[b * 4 + 0]["out"].astype(np.float32).copy()
        for g in range(1, 4):
            acc += res.results[b * 4 + g]["out"]
        out[b] = acc.T + const[None, :]
    return out


# revision 17
# speedup vs baseline: 2.4174x; 1.0171x over previous
"""Trainium2 Bass kernel for CausalCrossConditionalSelfAttention.

Reference semantics (B=2, T=2560, C=768, H=12, hd=64, t=T//10=256):
  q/k/v = x @ W{q,k,v}.T + b{q,k,v}           (per-head slices of C)
  att   = softmax(mask(q k^T / sqrt(hd)))      mask: (i%256) >= (j%256)
  y     = (att @ v) @ Wp.T + bp

Sharding: 8 cores = 2 batches x 4 head-groups (3 heads each).
Each core computes its (batch, 3 heads) slab fully on-chip and returns a
partial pre-projection output out^T [768, 2560]; the host sums the 4
head-group partials per batch and adds the constant bias (bp + Wp @ bv).

Device-side layout (per core):
  xT      [768, 2560]  x[b].T
  wqk     [768, 384]   cols: [Qh0|Qh1 | Kh0|Kh1 | Qh2 | Kh2] weight.T cols
  bqk     [4, 128, 1]  per-partition bias rows for the 4 col-groups
  wv      [768, 192]   Wv rows for the 3 heads, transposed
  wp      [3, 64, 768] per-head Wp[:, head_slice].T
  out     [768, 2560]  partial out^T (pre-bias)

The scores are computed transposed: S^T[k, q] in PSUM, exp'd on ScalarE
(scale=1/8 fused), masked by GPSIMD affine_select (exact zeros), and
contracted with V (ones column appended -> softmax denominator for free).
The (k%256)>=128 x (q%256)<128 quarter of each 256x256 mask block is fully
masked and skipped entirely (25% of score/AV/exp work).
"""

import numpy as np

B, T, C = 2, 2560, 768
H, HD = 12, 64
HPG = 3            # heads per group (core)
CW = HPG * HD      # 192
NKC = T // 128     # 20 key chunks of 128
NQT = T // 512     # 5 query tiles of 512
N_CORES = 8

_CACHE = {}


def _split_multi_waits(nc, maxw=1):
    """walrus in this container rejects >1 sync wait per instruction;
    split extra waits onto preceding NOPs on the same engine."""
    import concourse.mybir as mybir
    for f in nc.m.functions:
        for bb in f.blocks:
            newlist = []
            for ins in bb.instructions:
                si = ins.sync_info
                if si is not None and si.on_wait and len(si.on_wait) > maxw:
                    waits = list(si.on_wait)
                    chunks = [waits[i:i + maxw] for i in range(0, len(waits), maxw)]
                    for ch in chunks[:-1]:
                        newlist.append(mybir.InstNoOp(
                            name=f"WSPLIT-{nc.next_id()}",
                            engine=ins.engine,
                            sync_info=mybir.SyncInfo(on_wait=list(ch), on_update=[]),
                            text_hint="wait_split",
                        ))
                    ins.sync_info = mybir.SyncInfo(
                        on_wait=list(chunks[-1]), on_update=list(si.on_update))
                newlist.append(ins)
            bb.instructions = newlist
    return nc


def _chunks(lst, n):
    return [lst[i:i + n] for i in range(0, len(lst), n)]


def build_program():
    import concourse.bass as bass
    import concourse.mybir as mybir
    import concourse.tile as tile

    f32 = mybir.dt.float32
    bf16 = mybir.dt.bfloat16
    AF = mybir.ActivationFunctionType
    ALU = mybir.AluOpType

    nc = bass.Bass()
    xT = nc.dram_tensor("xT", [C, T], bf16, kind="ExternalInput")
    wqk = nc.dram_tensor("wqk", [C, 384], bf16, kind="ExternalInput")
    bqk = nc.dram_tensor("bqk", [4, 128, 1], f32, kind="ExternalInput")
    wv = nc.dram_tensor("wv", [C, CW], bf16, kind="ExternalInput")
    wp = nc.dram_tensor("wp", [HPG, HD, C], bf16, kind="ExternalInput")
    identm = nc.dram_tensor("identm", [128, 384], f32, kind="ExternalInput")
    out = nc.dram_tensor("out", [C, T], f32, kind="ExternalOutput")
    rcpb = nc.dram_tensor("rcpb", [HPG * NQT, 512], f32)

    with tile.TileContext(nc) as tc:
        with tc.tile_pool(name="persist", bufs=1) as persist, \
             tc.tile_pool(name="work", bufs=2) as work, \
             tc.tile_pool(name="psum", bufs=2, space="PSUM") as psum:

            # ---------------- load inputs ----------------
            wqk_sb = persist.tile([128, 6, 384], bf16)
            for c in range(6):
                nc.sync.dma_start(out=wqk_sb[:, c, :],
                                  in_=wqk[c * 128:(c + 1) * 128, :])
            wv_sb = persist.tile([128, 6, CW], bf16)
            for c in range(6):
                nc.sync.dma_start(out=wv_sb[:, c, :],
                                  in_=wv[c * 128:(c + 1) * 128, :])
            wp_sb = persist.tile([64, HPG, C], bf16)
            for h in range(HPG):
                nc.sync.dma_start(out=wp_sb[:, h, :], in_=wp[h])
            bqk_sb = persist.tile([128, 4, 1], f32)
            for j in range(4):
                nc.sync.dma_start(out=bqk_sb[:, j, :], in_=bqk[j])
            xt_sb = persist.tile([128, 6, T], bf16)       # x^T, 6 chunks of C
            for qt in range(NQT):
                for c in range(6):
                    nc.sync.dma_start(
                        out=xt_sb[:, c, qt * 512:(qt + 1) * 512],
                        in_=xT[c * 128:(c + 1) * 128, qt * 512:(qt + 1) * 512])

            ones_sb = persist.tile([128, 64], f32)
            nc.vector.memset(ones_sb, 1.0)
            identm_f = work.tile([128, 384], f32, tag="im", bufs=1, name="identm_f")
            nc.sync.dma_start(out=identm_f, in_=identm[:, :])
            ident_sb = persist.tile([128, 128], bf16)   # identity
            maskm_sb = persist.tile([128, 256], bf16)   # [L|L], L=-1e9 if j<i
            nc.vector.tensor_copy(ident_sb, identm_f[:, 0:128])
            nc.vector.tensor_copy(maskm_sb, identm_f[:, 128:384])

            # ---------------- q/k projections (transposed) ----------------
            # qkT j0=[Qh0|Qh1] j1=[Kh0|Kh1] (128 parts); j2=Qh2 j3=Kh2 (64)
            qkT01q = persist.tile([128, T], bf16)
            qkT01k = persist.tile([128, T], bf16)
            qkT2q = persist.tile([64, T], bf16)
            qkT2k = persist.tile([64, T], bf16)
            jdefs = [(qkT01q, 0, 128), (qkT01k, 128, 128),
                     (qkT2q, 256, 64), (qkT2k, 320, 64)]
            for qt in range(NQT):
                for j, (dst, col0, m) in enumerate(jdefs):
                    qk_ps = psum.tile([128, 512], f32, tag="av", name="qk_ps")
                    for c in range(6):
                        nc.tensor.matmul(
                            qk_ps[:m, :],
                            lhsT=wqk_sb[:, c, col0:col0 + m],
                            rhs=xt_sb[:, c, qt * 512:(qt + 1) * 512],
                            start=(c == 0), stop=(c == 5))
                    nc.vector.tensor_scalar_add(
                        dst[:m, qt * 512:(qt + 1) * 512],
                        qk_ps[:m, :], bqk_sb[:m, j, :])

            # ---------------- v projection (natural layout + ones col) ----
            # per head h: cols [65h .. 65h+63] = V_h, col 65h+64 = 1.0
            v_sb = persist.tile([128, NKC, HPG * 65], bf16)
            v_r = v_sb.rearrange("p n (h c) -> p n h c", c=65)
            nc.vector.memset(v_r[:, :, :, 64], 1.0)
            for tch in range(NKC):
                v_ps = psum.tile([128, 512], f32, tag="av", name="v_ps")
                for c in range(6):
                    nc.tensor.matmul(
                        v_ps[:, :CW],
                        lhsT=xt_sb[:, c, tch * 128:(tch + 1) * 128],
                        rhs=wv_sb[:, c, :],
                        start=(c == 0), stop=(c == 5))
                nc.vector.tensor_copy(
                    v_r[:, tch, :, 0:64],
                    v_ps[:, :CW].rearrange("p (h c) -> p h c", h=HPG))

            # ---------------- attention ----------------
            evens = list(range(0, NKC, 2))
            odds = list(range(1, NKC, 2))
            ynorm = [persist.tile([64, T], bf16, name=f"ynorm{h}")
                     for h in range(HPG)]

            pending = []

            def _emit_proj(qt_p):
                for m in range(6):
                    pj_ps = psum.tile([128, 512], f32, tag="sc", name="pj_ps")
                    for hh in range(HPG):
                        nc.tensor.matmul(
                            pj_ps,
                            lhsT=wp_sb[:, hh, m * 128:(m + 1) * 128],
                            rhs=ynorm[hh][:, qt_p * 512:(qt_p + 1) * 512],
                            start=(hh == 0), stop=(hh == 2))
                    pj_sb = work.tile([128, 512], f32, tag="pj", name="pj_sb")
                    nc.vector.tensor_copy(pj_sb, pj_ps)
                    nc.sync.dma_start(
                        out=out[m * 128:(m + 1) * 128,
                                qt_p * 512:(qt_p + 1) * 512],
                        in_=pj_sb)

            def _flush_norm(item):
                av_p, rcp_p, h_p, qt_p = item
                slot = h_p * NQT + qt_p
                bc_sb = work.tile([64, 512], f32, tag="bc", name="bc_sb")
                nc.sync.dma_start(out=rcpb[slot:slot+1, :], in_=rcp_p[64:65, :])
                bcast_in = bass.AP(tensor=rcpb, offset=slot * 512,
                                   ap=[[0, 64], [1, 512]])
                nc.sync.dma_start(out=bc_sb, in_=bcast_in)
                nc.vector.tensor_mul(
                    ynorm[h_p][:, qt_p * 512:(qt_p + 1) * 512],
                    av_p[0:64, :], bc_sb)
                if h_p == HPG - 1:
                    _emit_proj(qt_p)

            for qt in range(NQT):
                for h in range(HPG):
                    if h < 2:
                        qTh = qkT01q[64 * h:64 * (h + 1), :]
                        kTh = qkT01k[64 * h:64 * (h + 1), :]
                    else:
                        qTh = qkT2q[0:64, :]
                        kTh = qkT2k[0:64, :]
                    # odd-subchunk view of q: [64, qt, two, sp, 128]
                    q_odd = qTh.rearrange("p (q s t c) -> p q t s c",
                                          q=NQT, s=2, t=2, c=128)
                    qwin = qTh[:, qt * 512:(qt + 1) * 512]
                    av = psum.tile([128, 512], f32, tag="av", name="av")
                    av_odd = av.rearrange("p (s t c) -> p t s c",
                                          s=2, t=2, c=128)[:, 1]

                    for grp in _chunks(evens, 3):
                        L = len(grp)
                        sc = psum.tile([128, 1536], f32, tag="sc", name="sc")
                        for i, kc in enumerate(grp):
                            nc.tensor.matmul(
                                sc[:, i * 512:(i + 1) * 512],
                                lhsT=kTh[:, kc * 128:(kc + 1) * 128],
                                rhs=qwin, start=True, stop=False,
                                skip_group_check=True)
                        sc_r = sc.rearrange("p (l s t c) -> p l s t c",
                                            l=3, s=2, t=2, c=128)
                        for i in range(L):
                            nc.tensor.matmul(
                                sc_r[:, i, :, 0], lhsT=ident_sb, rhs=maskm_sb,
                                start=False, stop=True, skip_group_check=True)
                        pt = work.tile([128, 1536], bf16, tag="pt", name="pt")
                        nc.scalar.activation(pt[:, :L * 512], sc[:, :L * 512],
                                             AF.Exp, scale=0.125)
                        for i, kc in enumerate(grp):
                            nc.tensor.matmul(
                                av[:65, :],
                                lhsT=v_sb[:, kc, 65 * h:65 * h + 65],
                                rhs=pt[:, i * 512:(i + 1) * 512],
                                start=(kc == 0), stop=False,
                                skip_group_check=True)

                    if pending:
                        _flush_norm(pending.pop(0))
                    for gi, grp in enumerate(_chunks(odds, 3)):
                        L = len(grp)
                        last_grp = (gi == 3)
                        sc = psum.tile([128, 1536], f32, tag="sc", name="sc")
                        for i, kc in enumerate(grp):
                            # 256-wide blocks: two share a 2KB psum bank, and
                            # start=True zero-marks the WHOLE bank - only the
                            # first block of each bank may set it.
                            nc.tensor.matmul(
                                sc[:, i * 256:(i + 1) * 256],
                                lhsT=kTh[:, kc * 128:(kc + 1) * 128],
                                rhs=q_odd[:, qt, 1], start=(i % 2 == 0),
                                stop=False, skip_group_check=True)
                        for i in range(L):
                            nc.tensor.matmul(
                                sc[:, i * 256:(i + 1) * 256],
                                lhsT=ident_sb, rhs=maskm_sb,
                                start=False, stop=True, skip_group_check=True)
                        pt = work.tile([128, 1536], bf16, tag="pt", name="pt")
                        nc.scalar.activation(pt[:, :L * 256], sc[:, :L * 256],
                                             AF.Exp, scale=0.125)
                        for i, kc in enumerate(grp):
                            nc.tensor.matmul(
                                av_odd[:65],
                                lhsT=v_sb[:, kc, 65 * h:65 * h + 65],
                                rhs=pt[:, i * 256:(i + 1) * 256],
                                start=False, stop=(kc == NKC - 1),
                                skip_group_check=True)

                    # normalize: y = av[0:64] / av[64]  (denominator row)
                    rcp = work.tile([65, 512], f32, tag="rcp", name="rcp",
                                    bufs=3)
                    nc.vector.reciprocal(rcp[64:65, :], av[64:65, :])
                    pending.append((av, rcp, h, qt))

            while pending:
                _flush_norm(pending.pop(0))

    _split_multi_waits(nc)
    return nc


def get_program():
    if "nc" not in _CACHE:
        _CACHE["nc"] = build_program()
    return _CACHE["nc"]


def make_in_maps(x, Wk, bk, Wq, bq, Wv, bv, Wp, bp):
    x = np.asarray(x, dtype=np.float32)
    in_maps = []
    for core in range(N_CORES):
        b, g = divmod(core, 4)
        h0 = g * HPG
        r = slice(h0 * HD, (h0 + HPG) * HD)     # 192 head dims
        xt = np.ascontiguousarray(x[b].T)
        wq_g = np.asarray(Wq)[r]                 # [192, 768]
        wk_g = np.asarray(Wk)[r]
        # wqk cols: [Qh0|Qh1(128) | Kh0|Kh1(128) | Qh2(64) | Kh2(64)]
        wqk = np.concatenate(
            [wq_g[:128].T, wk_g[:128].T, wq_g[128:].T, wk_g[128:].T],
            axis=1).astype(np.float32)
        bq_g = np.asarray(bq)[r].astype(np.float32)
        bk_g = np.asarray(bk)[r].astype(np.float32)
        bqk = np.zeros((4, 128, 1), np.float32)
        bqk[0, :, 0] = bq_g[:128]
        bqk[1, :, 0] = bk_g[:128]
        bqk[2, :64, 0] = bq_g[128:]
        bqk[3, :64, 0] = bk_g[128:]
        wv_g = np.ascontiguousarray(np.asarray(Wv)[r].T).astype(np.float32)
        wp_g = np.asarray(Wp)[:, r]              # [768, 192]
        wp_t = np.ascontiguousarray(
            wp_g.T.reshape(HPG, HD, C)).astype(np.float32)
        ident = np.eye(128, dtype=np.float32)
        L = np.where(np.arange(256)[None, :] % 128 < np.arange(128)[:, None],
                     np.float32(-1e9), np.float32(0.0))
        identm = np.concatenate([ident, L], axis=1).astype(np.float32)
        import ml_dtypes
        b16 = ml_dtypes.bfloat16
        in_maps.append({
            "identm": identm,
            "xT": np.ascontiguousarray(xt).astype(b16),
            "wqk": np.ascontiguousarray(wqk).astype(b16),
            "bqk": bqk,
            "wv": wv_g.astype(b16),
            "wp": wp_t.astype(b16),
        })
    return in_maps


def kernel(x, Wk, bk, Wq, bq, Wv, bv, Wp, bp):
    from concourse.bass_utils import run_bass_kernel_spmd
    nc = get_program()
    in_maps = make_in_maps(x, Wk, bk, Wq, bq, Wv, bv, Wp, bp)
    res = run_bass_kernel_spmd(nc, in_maps, list(range(N_CORES)))
    Wp_np = np.asarray(Wp, dtype=np.float32)
    const = (np.asarray(bp, dtype=np.float32)
             + Wp_np @ np.asarray(bv, dtype=np.float32))   # [768]
    out = np.empty((B, T, C), dtype=np.float32)
    for b in range(B):
        acc = res.results[b * 4 + 0]["out"].astype(np.float32).copy()
        for g in range(1, 4):
            acc += res.results[b * 4 + g]["out"]
        out[b] = acc.T + const[None, :]
    return out
